# revision 17
# baseline (speedup 1.0000x reference)
"""Cross-attention Trainium2 kernel (nn_CrossAttention_52484500357469).

Shards the B*N = 8192 query rows across 8 NeuronCores (1024 rows each, no
cross-core communication; each core redundantly computes K/V for its batch).

Per-core algorithm (all layouts chosen so the PE contraction dim sits on
SBUF partitions and softmax needs no cross-partition reduction):
  xT   [320, 1024]   = PE-transpose of this core's x rows
  ctxT [768, 1024]   = PE-transpose of this batch's context
  qT   [512, 1024]   = Wq.T @ x.T      (e on partitions)
  kT   [512, 1024]   = Wk.T @ ctx.T
  v    [1024, 512]   = ctx @ Wv        (m on partitions), + ones column
  per head h:
    sT[m,n]  = kT_h-slices.T @ qT_h    (scoresT; m on partitions)
    expT     = exp(0.125 * sT)         (ACT, scale fused)
    oT|den   = [v_h | 1].T @ expT      (AV matmul; row 64 = softmax denom)
    oT_norm  = oT * (1/den)            (denom DMA-broadcast across partitions)
  y[n,:] = sum_h oT_norm_h.T @ Wo_h + bo

Matmul operands are float32r (full fp32 storage, single-pass PE) — the
compiler requires f32r-consumed tensors to be produced as f32r, so every
matmul input tile is written by an engine op with an f32r-typed output.
"""

import numpy as np

import concourse.bass as bass
import concourse.mybir as mybir
import concourse.tile as tile
from concourse import bacc, bass_utils

F32 = mybir.dt.float32
FR = mybir.dt.float32r

B, N, M = 2, 4096, 1024
QDIM, CDIM = 320, 768
H, DH = 8, 64
HD = H * DH  # 512
SCALE = DH ** -0.5
NCORES = 8
NLOC = B * N // NCORES  # 1024 query rows per core

_COMPILED = {}
_EXEC = {}


def _build_kernel(reps=1):
    nc = bacc.Bacc("TRN2", target_bir_lowering=False, debug=False,
                   num_devices=NCORES)

    x_d = nc.dram_tensor("x_loc", [NLOC, QDIM], F32, kind="ExternalInput")
    ctx_d = nc.dram_tensor("ctx", [M, CDIM], F32, kind="ExternalInput")
    wq_d = nc.dram_tensor("Wq", [QDIM, HD], F32, kind="ExternalInput")
    wk_d = nc.dram_tensor("Wk", [CDIM, HD], F32, kind="ExternalInput")
    wv_d = nc.dram_tensor("Wv", [CDIM, HD], F32, kind="ExternalInput")
    wo_d = nc.dram_tensor("Wo", [HD, QDIM], F32, kind="ExternalInput")
    bob_d = nc.dram_tensor("bo_bcast", [128, QDIM], F32, kind="ExternalInput")
    id_d = nc.dram_tensor("ident", [128, 128], F32, kind="ExternalInput")
    ones_d = nc.dram_tensor("ones_col", [128, 1], F32, kind="ExternalInput")
    out_d = nc.dram_tensor("out_loc", [NLOC, QDIM], F32, kind="ExternalOutput")

    with tile.TileContext(nc) as tc:
        args = (tc, x_d.ap(), ctx_d.ap(), wq_d.ap(), wk_d.ap(),
                wv_d.ap(), wo_d.ap(), bob_d.ap(), id_d.ap(),
                ones_d.ap(), out_d.ap())
        if reps > 1:
            # benchmark mode: repeat the (idempotent) body on-device so
            # per-iteration time can be extracted from wall-clock deltas
            with tc.For_i(0, reps, 1):
                _kernel_body(*args)
        else:
            _kernel_body(*args)

    nc.compile()
    return nc


def _kernel_body(tc, x, ctx, wq, wk, wv, wo, bob, ident_dram, ones_dram, out):
    nc = tc.nc
    NT = NLOC // 128   # 8 query row-tiles
    MT = M // 128      # 8 context row-tiles
    XC = 3             # QDIM chunks: 128, 128, 64
    CC = CDIM // 128   # 6
    EC = HD // 128     # 4

    with (
        tc.tile_pool(name="consts", bufs=1) as cpool,
        tc.tile_pool(name="qkv", bufs=1) as qkv,
    ):
        ident = cpool.tile([128, 128], F32, tag="ident")
        nc.sync.dma_start(ident[:], ident_dram)
        bo_sb = cpool.tile([128, QDIM], F32, tag="bo_sb")
        nc.sync.dma_start(bo_sb[:], bob)
        wo_sb = cpool.tile([64, H, QDIM], FR, tag="wo_sb")
        # ones row vector pinned at partition 64 (same partition as the AV
        # denominator row) to act as the broadcast matmul's stationary operand
        ones_r = cpool.tile([65, 64], FR, tag="ones_r")

        qT = qkv.tile([128, EC, NLOC], FR, tag="qT")
        kT = qkv.tile([128, EC, M], FR, tag="kT")
        v_sb = qkv.tile([128, MT, H, DH + 1], FR, tag="v_sb")
        oT_n = qkv.tile([64, H, NLOC], FR, tag="oT_n")

        # ---------------- phase 0/A: transposes + projections ----------------
        with (
            tc.tile_pool(name="wts", bufs=1) as wpool,
            tc.tile_pool(name="tr", bufs=1) as tpool,
            tc.tile_pool(name="ps_t", bufs=2, space="PSUM") as ps_t,
            tc.tile_pool(name="ps_a", bufs=3, space="PSUM") as ps_a,
        ):
            # fp32 staging + engine conversion to f32r (rounding the compiler
            # demands for f32r matmul operands); staging frees on scope exit
            with tc.tile_pool(name="wst", bufs=1) as wst:
                ones_st = wst.tile([128, 1], F32, tag="ones_st")
                nc.sync.dma_start(ones_st[:], ones_dram)
                nc.vector.tensor_copy(
                    out=v_sb[:, :, :, DH:DH + 1],
                    in_=ones_st.to_broadcast((128, MT, H, 1)))

                ones_rst = wst.tile([65, 64], F32, tag="ones_rst")
                nc.sync.dma_start(ones_rst[64:65, :],
                                  ones_dram.rearrange("p o -> o p")[0:1, 0:64])
                nc.scalar.copy(ones_r[64:65, :], ones_rst[64:65, :])

                wq_st = wst.tile([128, XC, HD], F32, tag="wq_st")
                nc.sync.dma_start(
                    wq_st[:, 0:2, :],
                    wq[0:256].rearrange("(c p) f -> p c f", p=128))
                nc.sync.dma_start(wq_st[0:64, 2, :], wq[256:320])
                wq_sb = wpool.tile([128, XC, HD], FR, tag="wq_sb")
                nc.vector.tensor_copy(out=wq_sb[:, 0:2, :], in_=wq_st[:, 0:2, :])
                nc.vector.tensor_copy(out=wq_sb[0:64, 2, :],
                                      in_=wq_st[0:64, 2, :])

                wk_st = wst.tile([128, CC, HD], F32, tag="wk_st")
                nc.sync.dma_start(wk_st[:],
                                  wk.rearrange("(c p) f -> p c f", p=128))
                wk_sb = wpool.tile([128, CC, HD], FR, tag="wk_sb")
                nc.scalar.copy(wk_sb[:], wk_st[:])

                wv_st = wst.tile([128, CC, HD], F32, tag="wv_st")
                nc.sync.dma_start(wv_st[:],
                                  wv.rearrange("(c p) f -> p c f", p=128))
                wv_sb = wpool.tile([128, CC, HD], FR, tag="wv_sb")
                nc.scalar.copy(wv_sb[:], wv_st[:])

                wo_st = wst.tile([64, H, QDIM], F32, tag="wo_st")
                nc.sync.dma_start(wo_st[:],
                                  wo.rearrange("(h p) f -> p h f", p=DH))
                nc.vector.tensor_copy(out=wo_sb[:], in_=wo_st[:])

            # transpose x and ctx via PE; inputs stream in as [128, cw]
            # column-slice tiles so SBUF holds only a few at a time
            with tc.tile_pool(name="ins", bufs=6) as ipool:
                xT = tpool.tile([128, XC, NLOC], FR, tag="xT")
                for c in range(XC):
                    cw = 128 if c < 2 else 64
                    for hf in range(NT // 4):
                        ps = ps_t.tile([128, 512], F32, tag="ps_t")
                        for g in range(4):
                            rt = hf * 4 + g
                            xt = ipool.tile([128, 128], F32, tag="in_t",
                                            name="xt")
                            nc.sync.dma_start(
                                xt[:, 0:cw],
                                x[rt * 128:(rt + 1) * 128,
                                  c * 128:c * 128 + cw])
                            nc.tensor.transpose(
                                ps[0:cw, g * 128:(g + 1) * 128],
                                xt[:, 0:cw], ident[:])
                        nc.scalar.copy(xT[0:cw, c, hf * 512:(hf + 1) * 512],
                                       ps[0:cw, :])

                ctxT = tpool.tile([128, CC, M], FR, tag="ctxT")
                for c in range(CC):
                    for hf in range(MT // 4):
                        ps = ps_t.tile([128, 512], F32, tag="ps_t")
                        for g in range(4):
                            rt = hf * 4 + g
                            ct = ipool.tile([128, 128], F32, tag="in_t",
                                            name="ct")
                            nc.sync.dma_start(
                                ct[:],
                                ctx[rt * 128:(rt + 1) * 128,
                                    c * 128:(c + 1) * 128])
                            nc.tensor.transpose(
                                ps[:, g * 128:(g + 1) * 128],
                                ct[:], ident[:])
                        nc.scalar.copy(ctxT[:, c, hf * 512:(hf + 1) * 512],
                                       ps[:])

            # qT[e, n] += Wq[d, e].T-chunks @ xT[d, n]
            for ec in range(EC):
                for nh in range(NLOC // 512):
                    ps = ps_a.tile([128, 512], F32, tag="ps_a")
                    for c in range(XC):
                        cw = 128 if c < 2 else 64
                        nc.tensor.matmul(
                            ps[:],
                            wq_sb[0:cw, c, ec * 128:(ec + 1) * 128],
                            xT[0:cw, c, nh * 512:(nh + 1) * 512],
                            start=(c == 0), stop=(c == XC - 1))
                    if (ec + nh) % 2 == 0:
                        nc.vector.tensor_copy(
                            out=qT[:, ec, nh * 512:(nh + 1) * 512], in_=ps[:])
                    else:
                        nc.scalar.copy(qT[:, ec, nh * 512:(nh + 1) * 512], ps[:])

            # kT[e, m] += Wk[d, e].T-chunks @ ctxT[d, m]
            for ec in range(EC):
                for mh in range(M // 512):
                    ps = ps_a.tile([128, 512], F32, tag="ps_a")
                    for c in range(CC):
                        nc.tensor.matmul(
                            ps[:],
                            wk_sb[:, c, ec * 128:(ec + 1) * 128],
                            ctxT[:, c, mh * 512:(mh + 1) * 512],
                            start=(c == 0), stop=(c == CC - 1))
                    if (ec + mh) % 2 == 0:
                        nc.vector.tensor_copy(
                            out=kT[:, ec, mh * 512:(mh + 1) * 512], in_=ps[:])
                    else:
                        nc.scalar.copy(kT[:, ec, mh * 512:(mh + 1) * 512], ps[:])

            # v[m, e] += ctxT[d, m-chunk].T @ Wv[d, e]
            for mc in range(MT):
                ps = ps_a.tile([128, 512], F32, tag="ps_a")
                for c in range(CC):
                    nc.tensor.matmul(
                        ps[:],
                        ctxT[:, c, mc * 128:(mc + 1) * 128],
                        wv_sb[:, c, :],
                        start=(c == 0), stop=(c == CC - 1))
                nc.vector.tensor_copy(
                    out=v_sb[:, mc, :, 0:DH],
                    in_=ps.rearrange("p (h e) -> p h e", h=H))

        # ---------------- attention ----------------
        with (
            tc.tile_pool(name="et", bufs=3) as epool,
            tc.tile_pool(name="rd", bufs=2) as rpool,
            tc.tile_pool(name="ps_s", bufs=2, space="PSUM") as ps_sp,
            tc.tile_pool(name="ps_o", bufs=1, space="PSUM") as ps_op,
            tc.tile_pool(name="ps_b", bufs=1, space="PSUM") as ps_bp,
        ):
            for h in range(H):
                ec, pb = h // 2, (h % 2) * 64
                ps_o = ps_op.tile([65, NLOC], F32, tag="ps_o")
                for mc in range(MT):
                    ps_s = ps_sp.tile([128, NLOC], F32, tag="ps_s")
                    for nh in range(NLOC // 512):
                        nc.tensor.matmul(
                            ps_s[:, nh * 512:(nh + 1) * 512],
                            kT[pb:pb + 64, ec, mc * 128:(mc + 1) * 128],
                            qT[pb:pb + 64, ec, nh * 512:(nh + 1) * 512],
                            start=True, stop=True)
                    et = epool.tile([128, NLOC], FR, tag="et")
                    nc.scalar.activation(et[:], ps_s[:],
                                         mybir.ActivationFunctionType.Exp,
                                         scale=SCALE)
                    for nh in range(NLOC // 512):
                        nc.tensor.matmul(
                            ps_o[:, nh * 512:(nh + 1) * 512],
                            v_sb[:, mc, h, :],
                            et[:, nh * 512:(nh + 1) * 512],
                            start=(mc == 0), stop=(mc == MT - 1))
                # softmax denominator lives in ps_o row 64: reciprocal it
                # (partition 64), broadcast across partitions 0-63 with a
                # K=1 ones matmul, then normalize rows 0-63.
                drow = rpool.tile([65, NLOC], FR, tag="drow")
                with nc.allow_low_precision(
                        reason="f32r reciprocal row feeds broadcast matmul"):
                    nc.vector.reciprocal(drow[64:65, :], ps_o[64:65, :])
                ps_b = ps_bp.tile([64, NLOC], F32, tag="ps_b")
                for nh in range(NLOC // 512):
                    nc.tensor.matmul(
                        ps_b[:, nh * 512:(nh + 1) * 512],
                        ones_r[64:65, :],
                        drow[64:65, nh * 512:(nh + 1) * 512],
                        start=True, stop=True)
                rbc = rpool.tile([64, NLOC], F32, tag="rbc")
                nc.vector.tensor_copy(out=rbc[:], in_=ps_b[:])
                nc.vector.tensor_mul(out=oT_n[:, h, :], in0=ps_o[0:64, :],
                                     in1=rbc[:])

        # ---------------- output projection ----------------
        with (
            tc.tile_pool(name="yt", bufs=2) as ypool,
            tc.tile_pool(name="ps_y", bufs=2, space="PSUM") as ps_yp,
        ):
            for nt in range(NT):
                ps_y = ps_yp.tile([128, QDIM], F32, tag="ps_y")
                for h in range(H):
                    nc.tensor.matmul(
                        ps_y[:],
                        oT_n[:, h, nt * 128:(nt + 1) * 128],
                        wo_sb[:, h, :],
                        start=(h == 0), stop=(h == H - 1))
                y_t = ypool.tile([128, QDIM], F32, tag="y_t")
                nc.vector.tensor_add(out=y_t[:], in0=ps_y[:], in1=bo_sb[:])
                nc.sync.dma_start(out[nt * 128:(nt + 1) * 128, :], y_t[:])


def _shard(inputs):
    x = np.ascontiguousarray(np.asarray(inputs["x"], dtype=np.float32))
    context = np.ascontiguousarray(np.asarray(inputs["context"], np.float32))
    wq = np.ascontiguousarray(np.asarray(inputs["Wq"], np.float32))
    wk = np.ascontiguousarray(np.asarray(inputs["Wk"], np.float32))
    wv = np.ascontiguousarray(np.asarray(inputs["Wv"], np.float32))
    wo = np.ascontiguousarray(np.asarray(inputs["Wo"], np.float32))
    bo = np.asarray(inputs["bo"], np.float32)

    xf = x.reshape(B * N, QDIM)
    bob = np.ascontiguousarray(np.broadcast_to(bo[None, :], (128, QDIM)))
    ident = np.eye(128, dtype=np.float32)
    ones_col = np.ones((128, 1), dtype=np.float32)

    in_maps = []
    for i in range(NCORES):
        b = (i * NLOC) // N
        in_maps.append({
            "x_loc": np.ascontiguousarray(xf[i * NLOC:(i + 1) * NLOC]),
            "ctx": context[b],
            "Wq": wq, "Wk": wk, "Wv": wv, "Wo": wo,
            "bo_bcast": bob, "ident": ident, "ones_col": ones_col,
        })
    return in_maps


def _get_exec(reps=1):
    """Compile (once) and wrap the 8-core SPMD executable with a cached jit."""
    import jax
    from jax.sharding import Mesh, PartitionSpec
    from jax.experimental.shard_map import shard_map
    import concourse.mybir as mb
    from concourse import bass2jax

    if reps in _EXEC:
        return _EXEC[reps]
    if reps not in _COMPILED:
        _COMPILED[reps] = _build_kernel(reps)
    nc = _COMPILED[reps]
    bass2jax.install_neuronx_cc_hook()

    partition_name = (nc.partition_id_tensor.name
                      if nc.partition_id_tensor else None)
    in_names, out_names, out_avals, zero_outs = [], [], [], []
    for alloc in nc.m.functions[0].allocations:
        if not isinstance(alloc, mb.MemoryLocationSet):
            continue
        name = alloc.memorylocations[0].name
        if alloc.kind == "ExternalInput":
            if name != partition_name:
                in_names.append(name)
        elif alloc.kind == "ExternalOutput":
            out_names.append(name)
            shape = tuple(alloc.tensor_shape)
            dtype = mb.dt.np(alloc.dtype)
            out_avals.append(jax.core.ShapedArray(shape, dtype))
            zero_outs.append(np.zeros(shape, dtype))
    n_params = len(in_names)
    all_names = in_names + out_names
    if partition_name is not None:
        all_names = all_names + [partition_name]

    def _body(*args):
        operands = list(args)
        if partition_name is not None:
            operands.append(bass2jax.partition_id_tensor())
        outs = bass2jax._bass_exec_p.bind(
            *operands,
            out_avals=tuple(out_avals),
            in_names=tuple(all_names),
            out_names=tuple(out_names),
            lowering_input_output_aliases=(),
            sim_require_finite=True,
            sim_require_nnan=True,
            nc=nc,
        )
        return tuple(outs)

    devices = jax.devices()[:NCORES]
    mesh = Mesh(np.asarray(devices), ("core",))
    n_outs = len(out_names)
    sharded = jax.jit(
        shard_map(_body, mesh=mesh,
                  in_specs=(PartitionSpec("core"),) * (n_params + n_outs),
                  out_specs=(PartitionSpec("core"),) * n_outs,
                  check_rep=False),
        donate_argnums=tuple(range(n_params, n_params + n_outs)),
        keep_unused=True)

    def run(in_maps):
        per_core = [[np.asarray(m[name]) for name in in_names]
                    for m in in_maps]
        concat_in = [
            np.concatenate([per_core[c][i] for c in range(NCORES)], axis=0)
            for i in range(n_params)
        ]
        concat_zeros = [
            np.zeros((NCORES * z.shape[0], *z.shape[1:]), z.dtype)
            for z in zero_outs
        ]
        out_arrs = sharded(*concat_in, *concat_zeros)
        return [
            {name: np.asarray(out_arrs[i]).reshape(NCORES,
                                                   *out_avals[i].shape)[c]
             for i, name in enumerate(out_names)}
            for c in range(NCORES)
        ]

    _EXEC[reps] = run
    return run


def _run(inputs, trace=False, reps=1):
    run = _get_exec(reps)
    results = run(_shard(inputs))
    parts = [np.asarray(r["out_loc"]) for r in results]
    full = np.concatenate(parts, axis=0).reshape(B, N, QDIM)
    return full.astype(np.float32), None


def kernel(**inputs):
    out, _ = _run(inputs)
    return out


# revision 18
# speedup vs baseline: 4.0754x; 4.0754x over previous
"""Cross-attention Trainium2 kernel (nn_CrossAttention_52484500357469).

Shards the B*N = 8192 query rows across 8 NeuronCores (1024 rows each, no
cross-core communication; each core redundantly computes K/V for its batch).

Per-core algorithm (all layouts chosen so the PE contraction dim sits on
SBUF partitions and softmax needs no cross-partition reduction):
  xT   [320, 1024]   = PE-transpose of this core's x rows
  ctxT [768, 1024]   = PE-transpose of this batch's context
  qT   [512, 1024]   = Wq.T @ x.T      (e on partitions)
  kT   [512, 1024]   = Wk.T @ ctx.T
  v    [1024, 512]   = ctx @ Wv        (m on partitions), + ones column
  per head h:
    sT[m,n]  = kT_h-slices.T @ qT_h    (scoresT; m on partitions)
    expT     = exp(0.125 * sT)         (ACT, scale fused)
    oT|den   = [v_h | 1].T @ expT      (AV matmul; row 64 = softmax denom)
    oT_norm  = oT * bcast(1/den)       (K=1 ones matmul broadcast)
  y[n,:] = sum_h oT_norm_h.T @ Wo_h + bo

Matmul operands are float32r (fp32 storage, single-pass PE) — the compiler
requires f32r-consumed tensors to be produced as f32r, so every matmul
input tile is written by an engine op with an f32r-typed output.
"""

import numpy as np

import concourse.bass as bass
import concourse.mybir as mybir
import concourse.tile as tile
from concourse import bacc, bass_utils

F32 = mybir.dt.float32
FR = mybir.dt.float32r

B, N, M = 2, 4096, 1024
QDIM, CDIM = 320, 768
H, DH = 8, 64
HD = H * DH  # 512
SCALE = DH ** -0.5
NCORES = 8
NLOC = B * N // NCORES  # 1024 query rows per core

_COMPILED = {}
_EXEC = {}


def _build_kernel(reps=1):
    nc = bacc.Bacc("TRN2", target_bir_lowering=False, debug=False,
                   num_devices=NCORES)

    x_d = nc.dram_tensor("x_loc", [NLOC, QDIM], F32, kind="ExternalInput")
    ctx_d = nc.dram_tensor("ctx", [M, CDIM], F32, kind="ExternalInput")
    wq_d = nc.dram_tensor("Wq", [QDIM, HD], F32, kind="ExternalInput")
    wk_d = nc.dram_tensor("Wk", [CDIM, HD], F32, kind="ExternalInput")
    wv_d = nc.dram_tensor("Wv", [CDIM, HD], F32, kind="ExternalInput")
    wo_d = nc.dram_tensor("Wo", [HD, QDIM], F32, kind="ExternalInput")
    # consts = [ident(128) | bo_bcast(320) | ones(1)] packed host-side
    cn_d = nc.dram_tensor("consts", [128, 449], F32, kind="ExternalInput")
    out_d = nc.dram_tensor("out_loc", [NLOC, QDIM], F32, kind="ExternalOutput")

    with tile.TileContext(nc) as tc:
        args = (tc, x_d.ap(), ctx_d.ap(), wq_d.ap(), wk_d.ap(),
                wv_d.ap(), wo_d.ap(), cn_d.ap(), out_d.ap())
        if reps > 1:
            # benchmark mode: repeat the (idempotent) body on-device so
            # per-iteration time can be extracted from wall-clock deltas
            with tc.For_i(0, reps, 1):
                _kernel_body(*args)
        else:
            _kernel_body(*args)

    nc.compile()
    return nc


def _kernel_body(tc, x, ctx, wq, wk, wv, wo, cn, out):
    nc = tc.nc
    NT = NLOC // 128   # 8 query row-tiles
    MT = M // 128      # 8 context row-tiles
    XC = 3             # QDIM chunks: 128, 128, 64
    CC = CDIM // 128   # 6
    EC = HD // 128     # 4

    with (
        tc.tile_pool(name="consts", bufs=1) as cpool,
        tc.tile_pool(name="qkv", bufs=1) as qkv,
    ):
        cn_sb = cpool.tile([128, 449], F32, tag="cn_sb")
        nc.sync.dma_start(cn_sb[:], cn)
        ident = cn_sb[:, 0:128]
        bo_sb = cn_sb[:, 128:448]
        ones_c = cn_sb[:, 448:449]
        wo_sb = cpool.tile([64, H, QDIM], FR, tag="wo_sb")
        # ones row vector pinned at partition 64 (same partition as the AV
        # denominator row) to act as the broadcast matmul's stationary operand
        ones_r = cpool.tile([65, 64], FR, tag="ones_r")

        qT = qkv.tile([128, EC, NLOC], FR, tag="qT")
        kT = qkv.tile([128, EC, M], FR, tag="kT")
        v_sb = qkv.tile([128, MT, H, DH + 1], FR, tag="v_sb")
        oT_n = qkv.tile([64, H, NLOC], FR, tag="oT_n")

        # ---------------- phase 0/A: transposes + projections ----------------
        with (
            tc.tile_pool(name="wts", bufs=1) as wpool,
            tc.tile_pool(name="tr", bufs=1) as tpool,
            tc.tile_pool(name="ps_t", bufs=4, space="PSUM") as ps_t,
            tc.tile_pool(name="ps_a", bufs=3, space="PSUM") as ps_a,
        ):
            # fp32 staging + engine conversion to f32r (the rounding the
            # compiler demands for f32r matmul operands); staging frees on
            # scope exit.  Weight DMAs ride the ACT HWDGE ring, x/ctx rows
            # the SP ring, so the two streams don't serialize.
            with tc.tile_pool(name="wst", bufs=1) as wst:
                nc.vector.tensor_copy(
                    out=v_sb[:, :, :, DH:DH + 1],
                    in_=ones_c.to_broadcast((128, MT, H, 1)))
                nc.scalar.copy(
                    ones_r[64:65, :],
                    cn_sb[64:65, 448:449].to_broadcast((1, 64)))

                wq_st = wst.tile([128, XC, HD], F32, tag="wq_st")
                nc.scalar.dma_start(
                    wq_st[:, 0:2, :],
                    wq[0:256].rearrange("(c p) f -> p c f", p=128))
                nc.scalar.dma_start(wq_st[0:64, 2, :], wq[256:320])
                wq_sb = wpool.tile([128, XC, HD], FR, tag="wq_sb")
                nc.vector.tensor_copy(out=wq_sb[:, 0:2, :],
                                      in_=wq_st[:, 0:2, :])
                nc.vector.tensor_copy(out=wq_sb[0:64, 2, :],
                                      in_=wq_st[0:64, 2, :])

                wk_st = wst.tile([128, CC, HD], F32, tag="wk_st")
                nc.scalar.dma_start(wk_st[:],
                                    wk.rearrange("(c p) f -> p c f", p=128))
                wk_sb = wpool.tile([128, CC, HD], FR, tag="wk_sb")
                nc.scalar.copy(wk_sb[:], wk_st[:])

                wv_st = wst.tile([128, CC, HD], F32, tag="wv_st")
                nc.scalar.dma_start(wv_st[:],
                                    wv.rearrange("(c p) f -> p c f", p=128))
                wv_sb = wpool.tile([128, CC, HD], FR, tag="wv_sb")
                nc.scalar.copy(wv_sb[:], wv_st[:])

                wo_st = wst.tile([64, H, QDIM], F32, tag="wo_st")
                nc.scalar.dma_start(wo_st[:],
                                    wo.rearrange("(h p) f -> p h f", p=DH))
                nc.vector.tensor_copy(out=wo_sb[:], in_=wo_st[:])

            # transpose x and ctx via PE, one full-row tile per DMA;
            # batch the per-row transposes into one psum tile and copy out
            # with a strided destination covering all chunks at once
            with tc.tile_pool(name="ins", bufs=3) as ipool:
                xT = tpool.tile([128, XC, NLOC], FR, tag="xT")
                for rt in range(NT):
                    xt = ipool.tile([128, QDIM], F32, tag="x_t", name="xt")
                    nc.sync.dma_start(xt[:],
                                      x[rt * 128:(rt + 1) * 128, :])
                    ps = ps_t.tile([128, 512], F32, tag="ps_t")
                    nc.tensor.transpose(ps[:, 0:128], xt[:, 0:128], ident)
                    nc.tensor.transpose(ps[:, 128:256], xt[:, 128:256], ident)
                    nc.tensor.transpose(ps[0:64, 256:384], xt[:, 256:320],
                                        ident)
                    rs = slice(rt * 128, (rt + 1) * 128)
                    nc.scalar.copy(
                        xT[:, 0:2, rs],
                        ps[:, 0:256].rearrange("p (c n) -> p c n", c=2))
                    nc.scalar.copy(xT[0:64, 2, rs], ps[0:64, 256:384])

                ctxT = tpool.tile([128, CC, M], FR, tag="ctxT")
                for rt in range(MT):
                    ct = ipool.tile([128, CDIM], F32, tag="c_t", name="ct")
                    nc.sync.dma_start(ct[:],
                                      ctx[rt * 128:(rt + 1) * 128, :])
                    ps = ps_t.tile([128, 512], F32, tag="ps_t")
                    for c in range(4):
                        nc.tensor.transpose(
                            ps[:, c * 128:(c + 1) * 128],
                            ct[:, c * 128:(c + 1) * 128], ident)
                    ps2 = ps_t.tile([128, 512], F32, tag="ps_t")
                    for c in range(2):
                        nc.tensor.transpose(
                            ps2[:, c * 128:(c + 1) * 128],
                            ct[:, (c + 4) * 128:(c + 5) * 128], ident)
                    rs = slice(rt * 128, (rt + 1) * 128)
                    nc.scalar.copy(
                        ctxT[:, 0:4, rs],
                        ps.rearrange("p (c n) -> p c n", c=4))
                    nc.scalar.copy(
                        ctxT[:, 4:6, rs],
                        ps2[:, 0:256].rearrange("p (c n) -> p c n", c=2))

            # qT[e, n] += Wq[d, e].T-chunks @ xT[d, n]
            for ec in range(EC):
                for nh in range(NLOC // 512):
                    ps = ps_a.tile([128, 512], F32, tag="ps_a")
                    for c in range(XC):
                        cw = 128 if c < 2 else 64
                        nc.tensor.matmul(
                            ps[:],
                            wq_sb[0:cw, c, ec * 128:(ec + 1) * 128],
                            xT[0:cw, c, nh * 512:(nh + 1) * 512],
                            start=(c == 0), stop=(c == XC - 1))
                    if (ec + nh) % 2 == 0:
                        nc.vector.tensor_copy(
                            out=qT[:, ec, nh * 512:(nh + 1) * 512], in_=ps[:])
                    else:
                        nc.scalar.copy(qT[:, ec, nh * 512:(nh + 1) * 512],
                                       ps[:])

            # kT[e, m] += Wk[d, e].T-chunks @ ctxT[d, m]
            for ec in range(EC):
                for mh in range(M // 512):
                    ps = ps_a.tile([128, 512], F32, tag="ps_a")
                    for c in range(CC):
                        nc.tensor.matmul(
                            ps[:],
                            wk_sb[:, c, ec * 128:(ec + 1) * 128],
                            ctxT[:, c, mh * 512:(mh + 1) * 512],
                            start=(c == 0), stop=(c == CC - 1))
                    if (ec + mh) % 2 == 0:
                        nc.vector.tensor_copy(
                            out=kT[:, ec, mh * 512:(mh + 1) * 512], in_=ps[:])
                    else:
                        nc.scalar.copy(kT[:, ec, mh * 512:(mh + 1) * 512],
                                       ps[:])

            # v[m, e] += ctxT[d, m-chunk].T @ Wv[d, e]
            for mc in range(MT):
                ps = ps_a.tile([128, 512], F32, tag="ps_a")
                for c in range(CC):
                    nc.tensor.matmul(
                        ps[:],
                        ctxT[:, c, mc * 128:(mc + 1) * 128],
                        wv_sb[:, c, :],
                        start=(c == 0), stop=(c == CC - 1))
                nc.vector.tensor_copy(
                    out=v_sb[:, mc, :, 0:DH],
                    in_=ps.rearrange("p (h e) -> p h e", h=H))

        # ---------------- attention ----------------
        with (
            tc.tile_pool(name="et", bufs=3) as epool,
            tc.tile_pool(name="rd", bufs=2) as rpool,
            tc.tile_pool(name="ps_s", bufs=2, space="PSUM") as ps_sp,
            tc.tile_pool(name="ps_o", bufs=1, space="PSUM") as ps_op,
            tc.tile_pool(name="ps_b", bufs=1, space="PSUM") as ps_bp,
        ):
            for h in range(H):
                ec, pb = h // 2, (h % 2) * 64
                ps_o = ps_op.tile([65, NLOC], F32, tag="ps_o")
                for mc in range(MT):
                    ps_s = ps_sp.tile([128, NLOC], F32, tag="ps_s")
                    for nh in range(NLOC // 512):
                        nc.tensor.matmul(
                            ps_s[:, nh * 512:(nh + 1) * 512],
                            kT[pb:pb + 64, ec, mc * 128:(mc + 1) * 128],
                            qT[pb:pb + 64, ec, nh * 512:(nh + 1) * 512],
                            start=True, stop=True)
                    et = epool.tile([128, NLOC], FR, tag="et")
                    nc.scalar.activation(et[:], ps_s[:],
                                         mybir.ActivationFunctionType.Exp,
                                         scale=SCALE)
                    for nh in range(NLOC // 512):
                        nc.tensor.matmul(
                            ps_o[:, nh * 512:(nh + 1) * 512],
                            v_sb[:, mc, h, :],
                            et[:, nh * 512:(nh + 1) * 512],
                            start=(mc == 0), stop=(mc == MT - 1))
                # softmax denominator lives in ps_o row 64: reciprocal it
                # (partition 64), broadcast across partitions 0-63 with a
                # K=1 ones matmul, then normalize rows 0-63.
                drow = rpool.tile([65, NLOC], FR, tag="drow")
                with nc.allow_low_precision(
                        reason="f32r reciprocal row feeds broadcast matmul"):
                    nc.vector.reciprocal(drow[64:65, :], ps_o[64:65, :])
                ps_b = ps_bp.tile([64, NLOC], F32, tag="ps_b")
                for nh in range(NLOC // 512):
                    nc.tensor.matmul(
                        ps_b[:, nh * 512:(nh + 1) * 512],
                        ones_r[64:65, :],
                        drow[64:65, nh * 512:(nh + 1) * 512],
                        start=True, stop=True)
                rbc = rpool.tile([64, NLOC], F32, tag="rbc")
                nc.vector.tensor_copy(out=rbc[:], in_=ps_b[:])
                nc.vector.tensor_mul(out=oT_n[:, h, :], in0=ps_o[0:64, :],
                                     in1=rbc[:])

        # ---------------- output projection ----------------
        with (
            tc.tile_pool(name="yt", bufs=2) as ypool,
            tc.tile_pool(name="ps_y", bufs=2, space="PSUM") as ps_yp,
        ):
            for nt in range(NT):
                ps_y = ps_yp.tile([128, QDIM], F32, tag="ps_y")
                for h in range(H):
                    nc.tensor.matmul(
                        ps_y[:],
                        oT_n[:, h, nt * 128:(nt + 1) * 128],
                        wo_sb[:, h, :],
                        start=(h == 0), stop=(h == H - 1))
                y_t = ypool.tile([128, QDIM], F32, tag="y_t")
                nc.vector.tensor_add(out=y_t[:], in0=ps_y[:], in1=bo_sb)
                nc.sync.dma_start(out[nt * 128:(nt + 1) * 128, :], y_t[:])


def _shard(inputs):
    x = np.ascontiguousarray(np.asarray(inputs["x"], dtype=np.float32))
    context = np.ascontiguousarray(np.asarray(inputs["context"], np.float32))
    wq = np.ascontiguousarray(np.asarray(inputs["Wq"], np.float32))
    wk = np.ascontiguousarray(np.asarray(inputs["Wk"], np.float32))
    wv = np.ascontiguousarray(np.asarray(inputs["Wv"], np.float32))
    wo = np.ascontiguousarray(np.asarray(inputs["Wo"], np.float32))
    bo = np.asarray(inputs["bo"], np.float32)

    xf = x.reshape(B * N, QDIM)
    consts = np.concatenate([
        np.eye(128, dtype=np.float32),
        np.broadcast_to(bo[None, :], (128, QDIM)),
        np.ones((128, 1), np.float32),
    ], axis=1)
    consts = np.ascontiguousarray(consts)

    in_maps = []
    for i in range(NCORES):
        b = (i * NLOC) // N
        in_maps.append({
            "x_loc": np.ascontiguousarray(xf[i * NLOC:(i + 1) * NLOC]),
            "ctx": context[b],
            "Wq": wq, "Wk": wk, "Wv": wv, "Wo": wo,
            "consts": consts,
        })
    return in_maps


def _get_exec(reps=1):
    """Compile (once) and wrap the 8-core SPMD executable with a cached jit.

    Used by the local test/bench harnesses; the graded kernel() entry goes
    through bass_utils.run_bass_kernel_spmd, which adapts to both axon
    (PJRT) and native (NRT) environments.
    """
    import jax
    from jax.sharding import Mesh, PartitionSpec
    from jax.experimental.shard_map import shard_map
    import concourse.mybir as mb
    from concourse import bass2jax

    if reps in _EXEC:
        return _EXEC[reps]
    if reps not in _COMPILED:
        _COMPILED[reps] = _build_kernel(reps)
    nc = _COMPILED[reps]
    bass2jax.install_neuronx_cc_hook()

    partition_name = (nc.partition_id_tensor.name
                      if nc.partition_id_tensor else None)
    in_names, out_names, out_avals, zero_outs = [], [], [], []
    for alloc in nc.m.functions[0].allocations:
        if not isinstance(alloc, mb.MemoryLocationSet):
            continue
        name = alloc.memorylocations[0].name
        if alloc.kind == "ExternalInput":
            if name != partition_name:
                in_names.append(name)
        elif alloc.kind == "ExternalOutput":
            out_names.append(name)
            shape = tuple(alloc.tensor_shape)
            dtype = mb.dt.np(alloc.dtype)
            out_avals.append(jax.core.ShapedArray(shape, dtype))
            zero_outs.append(np.zeros(shape, dtype))
    n_params = len(in_names)
    all_names = in_names + out_names
    if partition_name is not None:
        all_names = all_names + [partition_name]

    def _body(*args):
        operands = list(args)
        if partition_name is not None:
            operands.append(bass2jax.partition_id_tensor())
        outs = bass2jax._bass_exec_p.bind(
            *operands,
            out_avals=tuple(out_avals),
            in_names=tuple(all_names),
            out_names=tuple(out_names),
            lowering_input_output_aliases=(),
            sim_require_finite=True,
            sim_require_nnan=True,
            nc=nc,
        )
        return tuple(outs)

    devices = jax.devices()[:NCORES]
    mesh = Mesh(np.asarray(devices), ("core",))
    n_outs = len(out_names)
    sharded = jax.jit(
        shard_map(_body, mesh=mesh,
                  in_specs=(PartitionSpec("core"),) * (n_params + n_outs),
                  out_specs=(PartitionSpec("core"),) * n_outs,
                  check_rep=False),
        donate_argnums=tuple(range(n_params, n_params + n_outs)),
        keep_unused=True)

    def run(in_maps):
        per_core = [[np.asarray(m[name]) for name in in_names]
                    for m in in_maps]
        concat_in = [
            np.concatenate([per_core[c][i] for c in range(NCORES)], axis=0)
            for i in range(n_params)
        ]
        concat_zeros = [
            np.zeros((NCORES * z.shape[0], *z.shape[1:]), z.dtype)
            for z in zero_outs
        ]
        out_arrs = sharded(*concat_in, *concat_zeros)
        return [
            {name: np.asarray(out_arrs[i]).reshape(NCORES,
                                                   *out_avals[i].shape)[c]
             for i, name in enumerate(out_names)}
            for c in range(NCORES)
        ]

    _EXEC[reps] = run
    return run


def _run(inputs, trace=False, reps=1):
    run = _get_exec(reps)
    results = run(_shard(inputs))
    parts = [np.asarray(r["out_loc"]) for r in results]
    full = np.concatenate(parts, axis=0).reshape(B, N, QDIM)
    return full.astype(np.float32), None


def kernel(**inputs):
    if 1 not in _COMPILED:
        _COMPILED[1] = _build_kernel(1)
    nc = _COMPILED[1]
    res = bass_utils.run_bass_kernel_spmd(
        nc, _shard(inputs), core_ids=list(range(NCORES)))
    parts = [np.asarray(r["out_loc"]) for r in res.results]
    return np.concatenate(parts, axis=0).reshape(B, N, QDIM).astype(np.float32)
